# revision 34
# baseline (speedup 1.0000x reference)
"""DeepseekOCR text MoE layer on 8 Trainium2 NeuronCores.

Expert-parallel with a unified slot structure: each core runs FIVE
identical expert cycles — 4 routed experts (bucketed by token count so
every core's slot has a similar load) plus the shared expert as a 5th
slot. The shared expert is 2D-sharded (4 token-quarters x 2 halves of
its 2816-wide intermediate): each core computes one (quarter, half)
cell = 512 tokens x 1408 cols = exactly 11 partition tiles, so the
352-column padding waste of 1D sharding disappears and the shared
weights STREAM through the same pools as expert weights (no residency).

Device program per core, per slot s (C = slot token capacity):
  phase A:  hT[h,c] = silu(wg.T @ xT) * (wu.T @ xT)   (16 k-tile acc)
  phase B:  yT[d,c] = wd.T-tiles @ hT                 (11 h-tile acc)
Host: routed slots scatter-add yT * combine_w; shared slot adds yT.T
into its token quarter (2 cores per quarter, halves sum).

Engine discipline: all loads issue on sync, emission-ordered so a
semaphore-gated load never sits ahead of an earlier-needed one; stores
+ silu on scalar, merged 4 output tiles per store so trailing stores
never delay a silu. B-phase PSUM groups alternate across both pools
(8 banks of elasticity against store-completion lag).
"""

import numpy as np
import ml_dtypes

import concourse.bacc as bacc
import concourse.mybir as mybir
import concourse.tile as tile
from concourse.bass_utils import run_bass_kernel_spmd

B, S, D = 2, 1024, 2048
E, H, K = 32, 1408, 6
H_SHARED = 2816
ROUTED_SCALE = 1.0
T = B * S                      # 2048 tokens
N_CORES = 8
E_LOC = E // N_CORES           # 4 routed experts per core
NSLOT = E_LOC + 1              # + shared-expert slot
SH_POS = 1                     # shared slot position in the cycle order
HS_LOC = H_SHARED // 2         # 1408 shared cols per core (2-way split)
TQ = 512                       # shared-expert token quarter
NH = H // 128                  # 11 h-tiles per slot (routed == shared half)
ND = D // 512                  # 4 d-groups (512 cols each)
NKD = D // 128                 # 16 contraction k-tiles over D
NXG = 4                        # x split into k-chunks for early start
KC = NKD // NXG                # 4 k-tiles per chunk

BF16 = ml_dtypes.bfloat16
f32 = mybir.dt.float32
bf16 = mybir.dt.bfloat16

LAST_RESULTS = None            # BassKernelResults of the latest run (for test harness)


def _route(x, gate_w):
    """Greedy top-k softmax router, fp32 numpy (matches jax.lax.top_k order)."""
    logits = x @ gate_w.T                              # [T, E]
    m = logits.max(-1, keepdims=True)
    ex = np.exp(logits - m)
    scores = ex / ex.sum(-1, keepdims=True)
    topk_i = np.argsort(-scores, axis=-1, kind="stable")[:, :K]
    topk_w = np.take_along_axis(scores, topk_i, -1) * ROUTED_SCALE
    return topk_i, topk_w.astype(np.float32)


def _build_bass(Cs):
    """Per-core Tile program; Cs[s] = token capacity of slot s (len NSLOT)."""
    nc = bacc.Bacc(None, target_bir_lowering=False)

    xgt = [nc.dram_tensor(f"xgt{s}", [128, NKD, Cs[s]], bf16, kind="ExternalInput")
           for s in range(NSLOT)]
    wgu = nc.dram_tensor("wgu", [NSLOT, NH, 128, 2, NKD, 128], bf16,
                         kind="ExternalInput")
    wdd = nc.dram_tensor("wdd", [NSLOT, 2 * ND, 128, NH, 2, 128], bf16,
                         kind="ExternalInput")
    y_out = [nc.dram_tensor(f"y_out{s}", [ND, 128, 4, Cs[s]], bf16,
                            kind="ExternalOutput")
             for s in range(NSLOT)]

    with tile.TileContext(nc) as tc:
        with (
            tc.tile_pool(name="wgu_p", bufs=10) as wgu_p,
            tc.tile_pool(name="wd_p", bufs=8) as wd_p,
            tc.tile_pool(name="xg_p", bufs=2 * NXG) as xg_p,
            tc.tile_pool(name="ht_p", bufs=1) as ht_p,
            tc.tile_pool(name="tmp_p", bufs=2) as tmp_p,
            tc.tile_pool(name="y_p", bufs=6) as y_p,
            tc.tile_pool(name="psA", bufs=4, space="PSUM") as psA,
            tc.tile_pool(name="psB", bufs=4, space="PSUM") as psB,
        ):
            # PE warm-up on zeros while the first loads land (HAM un-throttle);
            # short tail MMs so the real stream starts promptly when data lands
            warm = tmp_p.tile([128, 512], bf16, tag="tmp")
            nc.vector.memset(warm[:], 0.0)
            pwarm = psA.tile([128, 512], f32, tag="psA")
            for _ in range(7):
                nc.tensor.matmul(pwarm[:], warm[:, :128], warm[:], start=True, stop=True)
            for _ in range(4):
                nc.tensor.matmul(pwarm[:, :128], warm[:, :128], warm[:, :128],
                                 start=True, stop=True)

            # ---- slot-0 critical loads, interleaved in consumption order on
            # ONE issue stream so the first MM group's operands arrive in the
            # order the k-loop needs ----
            gu0 = wgu_p.tile([128, 2, NKD, 128], bf16, tag="wgu", name="wgu0_h0")
            xg0 = []

            def _crit_gu(pr, ks):
                nc.sync.dma_start(gu0[:, pr, ks], wgu[0, 0, :, pr, ks])

            def _crit_xg(g):
                xc = xg_p.tile([128, KC, Cs[0]], bf16, tag="xg", name=f"xg0_{g}")
                nc.sync.dma_start(xc[:], xgt[0][:, g * KC:(g + 1) * KC, :])
                xg0.append(xc)

            _crit_gu(0, slice(0, 4)); _crit_xg(0)
            _crit_gu(0, slice(4, 8)); _crit_xg(1)
            _crit_gu(0, slice(8, 12)); _crit_gu(1, slice(0, 8)); _crit_xg(2)
            _crit_gu(0, slice(12, 16)); _crit_xg(3)
            _crit_gu(1, slice(8, 16))
            slabs0 = [(gu0[:, 0], gu0[:, 1])]
            for h in range(1, NH):
                gu = wgu_p.tile([128, 2, NKD, 128], bf16, tag="wgu")
                nc.sync.dma_start(gu[:], wgu[0, h])
                slabs0.append((gu[:, 0], gu[:, 1]))

            def load_xg(s):
                chunks = []
                for g in range(NXG):
                    xc = xg_p.tile([128, KC, Cs[s]], bf16, tag="xg", name=f"xg{s}_{g}")
                    nc.sync.dma_start(xc[:], xgt[s][:, g * KC:(g + 1) * KC, :])
                    chunks.append(xc)
                return chunks

            def load_wgu(s, h):
                gu = wgu_p.tile([128, 2, NKD, 128], bf16, tag="wgu", name=f"wgu{s}_h{h}")
                nc.sync.dma_start(gu[:], wgu[s, h])
                return (gu[:, 0], gu[:, 1])

            xg_cur, slabs_cur = xg0, slabs0
            for s in range(NSLOT):
                C = Cs[s]
                # ---- phase A: gate/up projections + silu*mul -> hT ----
                hT = ht_p.tile([128, NH, C], bf16, tag="ht")
                for h in range(NH):
                    wg_s, wu_s = slabs_cur[h]
                    pg = psA.tile([128, C], f32, tag="psA")
                    for k in range(NKD):
                        nc.tensor.matmul(pg[:], wg_s[:, k], xg_cur[k // KC][:, k % KC],
                                         start=(k == 0), stop=(k == NKD - 1))
                    pu = psA.tile([128, C], f32, tag="psA")
                    for k in range(NKD):
                        nc.tensor.matmul(pu[:], wu_s[:, k], xg_cur[k // KC][:, k % KC],
                                         start=(k == 0), stop=(k == NKD - 1))
                    tmp = tmp_p.tile([128, 512], bf16, tag="tmp")
                    nc.scalar.activation(tmp[:, :C], pg[:],
                                         mybir.ActivationFunctionType.Silu)
                    nc.vector.tensor_mul(hT[:, h, :], tmp[:, :C], pu[:])

                # next slot's tokens, THIS slot's down-proj weights (ungated:
                # buffers freed in B(s-1), so the stream runs during A(s)),
                # then the next slot's weight slabs (gates spread over A(s))
                if s + 1 < NSLOT:
                    xg_cur = load_xg(s + 1)
                wd_slabs = []
                for dh in range(2 * ND):
                    wd_s = wd_p.tile([128, NH, 2, 128], bf16, tag="wd")
                    nc.sync.dma_start(wd_s[:], wdd[s, dh])
                    wd_slabs.append(wd_s)
                if s + 1 < NSLOT:
                    slabs_cur = [load_wgu(s + 1, h) for h in range(9)]

                # ---- phase B: stationary = wd d-tiles, moving = hT tokens;
                # 4 consecutive (dh,dt) outputs merge into one store ----
                for qd in range(ND):
                    yst = y_p.tile([128, 4, C], bf16, tag="y")
                    split = (s == NSLOT - 1 and qd == ND - 1)
                    for g4 in range(4):
                        g = qd * 4 + g4
                        dh, dt = g // 2, g % 2
                        pp = psB if g % 2 == 0 else psA
                        py = pp.tile([128, 512], f32,
                                     tag="psB" if pp is psB else "psA")
                        for h in range(NH):
                            nc.tensor.matmul(py[:, :C], wd_slabs[dh][:, h, dt],
                                             hT[:, h, :],
                                             start=(h == 0), stop=(h == NH - 1))
                        nc.vector.tensor_copy(yst[:, g4], py[:, :C])
                        if split:
                            eng = nc.scalar if g4 % 2 == 0 else nc.sync
                            eng.dma_start(y_out[s][qd, :, g4], yst[:, g4])
                    if not split:
                        nc.scalar.dma_start(y_out[s][qd], yst[:])
                # late slabs for the next slot (gates clear in A(s+1))
                if s + 1 < NSLOT:
                    slabs_cur = slabs_cur + [load_wgu(s + 1, h)
                                             for h in range(9, NH)]
    nc.compile()
    return nc


def _wgu_pack(g, u):
    """[D,Hc] gate/up -> [NH, 128, 2, NKD, 128] slab layout."""
    return (np.stack([g, u]).reshape(2, NKD, 128, NH, 128)
            .transpose(3, 2, 0, 1, 4).astype(BF16))


def _wdd_pack(w):
    """[Hc,D] down -> [2*ND, 128, NH, 2, 128] slab layout."""
    return (w.reshape(NH, 128, 2 * ND, 2, 128)
            .transpose(2, 1, 0, 3, 4).astype(BF16))


def _xgt_pack(xg, C):
    """[D,cnt] gathered tokens -> [128, NKD, C] chunk layout."""
    cnt = xg.shape[1]
    out = np.zeros((128, NKD, C), BF16)
    out[:, :, :cnt] = xg.reshape(NKD, 128, cnt).transpose(1, 0, 2).astype(BF16)
    return np.ascontiguousarray(out)


def kernel(hidden_states, gate_w, wg, wu, wd, swg, swu, swd):
    global LAST_RESULTS
    x = np.ascontiguousarray(np.asarray(hidden_states, np.float32).reshape(T, D))
    gate_w = np.asarray(gate_w, np.float32)
    wg = np.asarray(wg, np.float32)
    wu = np.asarray(wu, np.float32)
    wd = np.asarray(wd, np.float32)
    swg = np.asarray(swg, np.float32)
    swu = np.asarray(swu, np.float32)
    swd = np.asarray(swd, np.float32)

    # ---- host router ----
    topk_i, topk_w = _route(x, gate_w)
    idx = [np.where((topk_i == e).any(-1))[0] for e in range(E)]
    wts = [(topk_w * (topk_i == e))[idx[e]].sum(-1).astype(np.float32) for e in range(E)]
    cnts = np.array([len(i) for i in idx])
    # bucket experts: slot j on every core serves similarly-loaded experts
    ranked = np.argsort(-cnts, kind="stable")            # expert ids, busiest first
    emap = ranked.reshape(E_LOC, N_CORES)                # emap[j, c] -> expert id
    Cr = [max(16, -(-int(cnts[emap[j]].max()) // 4) * 4) for j in range(E_LOC)]
    # cycle order: smallest routed slot first (smallest startup-critical
    # load), then shared, then the remaining routed slots
    jorder = [E_LOC - 1] + list(range(E_LOC - 1))
    Cs = [Cr[jorder[0]], TQ] + [Cr[j] for j in jorder[1:]]
    jmap = [jorder[0], None] + jorder[1:]

    nc = _build_bass(Cs)

    # ---- host shard + layout prep (all DMA sources partition-major) ----
    xT = np.ascontiguousarray(x.T)                      # [D, T] fp32

    in_maps = []
    for c in range(N_CORES):
        qc, hc = c >> 1, c & 1                          # shared (quarter, half)
        wgu_np = np.empty((NSLOT, NH, 128, 2, NKD, 128), BF16)
        wdd_np = np.empty((NSLOT, 2 * ND, 128, NH, 2, 128), BF16)
        imap = {"wgu": wgu_np, "wdd": wdd_np}
        hs = slice(hc * HS_LOC, (hc + 1) * HS_LOC)
        for s in range(NSLOT):
            j = jmap[s]
            if j is None:                               # shared-expert slot
                wgu_np[s] = _wgu_pack(swg[:, hs], swu[:, hs])
                wdd_np[s] = _wdd_pack(swd[hs, :])
                imap[f"xgt{s}"] = _xgt_pack(xT[:, qc * TQ:(qc + 1) * TQ], TQ)
            else:
                e = int(emap[j, c])
                wgu_np[s] = _wgu_pack(wg[e], wu[e])
                wdd_np[s] = _wdd_pack(wd[e])
                imap[f"xgt{s}"] = _xgt_pack(xT[:, idx[e]], Cs[s])
        in_maps.append(imap)

    res = run_bass_kernel_spmd(nc, in_maps, core_ids=list(range(N_CORES)))
    LAST_RESULTS = res

    # ---- host unshard: scatter-add routed outputs, add shared partials ----
    out = np.zeros((T, D), np.float32)
    for c in range(N_CORES):
        qc = c >> 1
        for s in range(NSLOT):
            y = (res.results[c][f"y_out{s}"]            # [ND, 128, 4, Cs[s]] bf16
                 .transpose(0, 2, 1, 3).reshape(D, Cs[s]).astype(np.float32))
            j = jmap[s]
            if j is None:
                out[qc * TQ:(qc + 1) * TQ] += y.T
            else:
                e = int(emap[j, c])
                cnt = int(cnts[e])
                out[idx[e]] += (y[:, :cnt] * wts[e][None, :]).T
    return out.reshape(B, S, D)


# revision 35
# speedup vs baseline: 1.0023x; 1.0023x over previous
"""DeepseekOCR text MoE layer on 8 Trainium2 NeuronCores.

Expert-parallel with a unified slot structure: each core runs FIVE
identical expert cycles — 4 routed experts (bucketed by token count so
every core's slot has a similar load) plus the shared expert as a 5th
slot. The shared expert is 2D-sharded (4 token-quarters x 2 halves of
its 2816-wide intermediate): each core computes one (quarter, half)
cell = 512 tokens x 1408 cols = exactly 11 partition tiles, so the
352-column padding waste of 1D sharding disappears and the shared
weights STREAM through the same pools as expert weights (no residency).

Device program per core, per slot s (C = slot token capacity):
  phase A:  hT[h,c] = silu(wg.T @ xT) * (wu.T @ xT)   (16 k-tile acc)
  phase B:  yT[d,c] = wd.T-tiles @ hT                 (11 h-tile acc)
Host: routed slots scatter-add yT * combine_w; shared slot adds yT.T
into its token quarter (2 cores per quarter, halves sum).

Engine discipline: all loads issue on sync, emission-ordered so a
semaphore-gated load never sits ahead of an earlier-needed one; stores
+ silu on scalar, merged 4 output tiles per store so trailing stores
never delay a silu. B-phase PSUM groups alternate across both pools
(8 banks of elasticity against store-completion lag).
"""

import numpy as np
import ml_dtypes

import concourse.bacc as bacc
import concourse.mybir as mybir
import concourse.tile as tile
from concourse.bass_utils import run_bass_kernel_spmd

B, S, D = 2, 1024, 2048
E, H, K = 32, 1408, 6
H_SHARED = 2816
ROUTED_SCALE = 1.0
T = B * S                      # 2048 tokens
N_CORES = 8
E_LOC = E // N_CORES           # 4 routed experts per core
NSLOT = E_LOC + 1              # + shared-expert slot
SH_POS = 1                     # shared slot position in the cycle order
HS_LOC = H_SHARED // 2         # 1408 shared cols per core (2-way split)
TQ = 512                       # shared-expert token quarter
NH = H // 128                  # 11 h-tiles per slot (routed == shared half)
ND = D // 512                  # 4 d-groups (512 cols each)
NKD = D // 128                 # 16 contraction k-tiles over D
NXG = 4                        # x split into k-chunks for early start
KC = NKD // NXG                # 4 k-tiles per chunk

BF16 = ml_dtypes.bfloat16
f32 = mybir.dt.float32
bf16 = mybir.dt.bfloat16

LAST_RESULTS = None            # BassKernelResults of the latest run (for test harness)


def _route(x, gate_w):
    """Greedy top-k softmax router, fp32 numpy (matches jax.lax.top_k order)."""
    logits = x @ gate_w.T                              # [T, E]
    m = logits.max(-1, keepdims=True)
    ex = np.exp(logits - m)
    scores = ex / ex.sum(-1, keepdims=True)
    topk_i = np.argsort(-scores, axis=-1, kind="stable")[:, :K]
    topk_w = np.take_along_axis(scores, topk_i, -1) * ROUTED_SCALE
    return topk_i, topk_w.astype(np.float32)


def _build_bass(Cs):
    """Per-core Tile program; Cs[s] = token capacity of slot s (len NSLOT)."""
    nc = bacc.Bacc(None, target_bir_lowering=False)

    xgt = [nc.dram_tensor(f"xgt{s}", [128, NKD, Cs[s]], bf16, kind="ExternalInput")
           for s in range(NSLOT)]
    wgu = nc.dram_tensor("wgu", [NSLOT, NH, 128, 2, NKD, 128], bf16,
                         kind="ExternalInput")
    wdd = nc.dram_tensor("wdd", [NSLOT, 2 * ND, 128, NH, 2, 128], bf16,
                         kind="ExternalInput")
    y_out = [nc.dram_tensor(f"y_out{s}", [ND, 128, 4, Cs[s]], bf16,
                            kind="ExternalOutput")
             for s in range(NSLOT)]

    with tile.TileContext(nc) as tc:
        with (
            tc.tile_pool(name="wgu_p", bufs=10) as wgu_p,
            tc.tile_pool(name="wd_p", bufs=8) as wd_p,
            tc.tile_pool(name="xg_p", bufs=2 * NXG) as xg_p,
            tc.tile_pool(name="ht_p", bufs=1) as ht_p,
            tc.tile_pool(name="tmp_p", bufs=2) as tmp_p,
            tc.tile_pool(name="y_p", bufs=6) as y_p,
            tc.tile_pool(name="psA", bufs=4, space="PSUM") as psA,
            tc.tile_pool(name="psB", bufs=4, space="PSUM") as psB,
        ):
            # PE warm-up on zeros while the first loads land (HAM un-throttle);
            # short tail MMs so the real stream starts promptly when data lands
            warm = tmp_p.tile([128, 512], bf16, tag="tmp")
            nc.vector.memset(warm[:], 0.0)
            pwarm = psA.tile([128, 512], f32, tag="psA")
            for _ in range(7):
                nc.tensor.matmul(pwarm[:], warm[:, :128], warm[:], start=True, stop=True)
            for _ in range(4):
                nc.tensor.matmul(pwarm[:, :128], warm[:, :128], warm[:, :128],
                                 start=True, stop=True)

            # ---- slot-0 critical loads, interleaved in consumption order on
            # ONE issue stream so the first MM group's operands arrive in the
            # order the k-loop needs ----
            gu0 = wgu_p.tile([128, 2, NKD, 128], bf16, tag="wgu", name="wgu0_h0")
            xg0 = []

            def _crit_gu(pr, ks):
                nc.sync.dma_start(gu0[:, pr, ks], wgu[0, 0, :, pr, ks])

            def _crit_xg(g):
                xc = xg_p.tile([128, KC, Cs[0]], bf16, tag="xg", name=f"xg0_{g}")
                nc.sync.dma_start(xc[:], xgt[0][:, g * KC:(g + 1) * KC, :])
                xg0.append(xc)

            _crit_gu(0, slice(0, 4)); _crit_xg(0)
            _crit_gu(0, slice(4, 8)); _crit_xg(1)
            _crit_gu(0, slice(8, 12)); _crit_gu(1, slice(0, 8)); _crit_xg(2)
            _crit_gu(0, slice(12, 16)); _crit_xg(3)
            _crit_gu(1, slice(8, 16))
            slabs0 = [(gu0[:, 0], gu0[:, 1])]
            for h in range(1, NH):
                gu = wgu_p.tile([128, 2, NKD, 128], bf16, tag="wgu")
                nc.sync.dma_start(gu[:], wgu[0, h])
                slabs0.append((gu[:, 0], gu[:, 1]))

            def load_xg(s):
                chunks = []
                for g in range(NXG):
                    xc = xg_p.tile([128, KC, Cs[s]], bf16, tag="xg", name=f"xg{s}_{g}")
                    nc.sync.dma_start(xc[:], xgt[s][:, g * KC:(g + 1) * KC, :])
                    chunks.append(xc)
                return chunks

            def load_wgu(s, h):
                gu = wgu_p.tile([128, 2, NKD, 128], bf16, tag="wgu", name=f"wgu{s}_h{h}")
                nc.sync.dma_start(gu[:], wgu[s, h])
                return (gu[:, 0], gu[:, 1])

            xg_cur, slabs_cur = xg0, slabs0
            for s in range(NSLOT):
                C = Cs[s]
                # ---- phase A: gate/up projections + silu*mul -> hT ----
                hT = ht_p.tile([128, NH, C], bf16, tag="ht")
                for h in range(NH):
                    wg_s, wu_s = slabs_cur[h]
                    pg = psA.tile([128, C], f32, tag="psA")
                    for k in range(NKD):
                        nc.tensor.matmul(pg[:], wg_s[:, k], xg_cur[k // KC][:, k % KC],
                                         start=(k == 0), stop=(k == NKD - 1))
                    pu = psA.tile([128, C], f32, tag="psA")
                    for k in range(NKD):
                        nc.tensor.matmul(pu[:], wu_s[:, k], xg_cur[k // KC][:, k % KC],
                                         start=(k == 0), stop=(k == NKD - 1))
                    tmp = tmp_p.tile([128, 512], bf16, tag="tmp")
                    nc.scalar.activation(tmp[:, :C], pg[:],
                                         mybir.ActivationFunctionType.Silu)
                    nc.vector.tensor_mul(hT[:, h, :], tmp[:, :C], pu[:])

                # next slot's tokens, THIS slot's down-proj weights (ungated:
                # buffers freed in B(s-1), so the stream runs during A(s)),
                # then the next slot's weight slabs (gates spread over A(s))
                if s + 1 < NSLOT:
                    xg_cur = load_xg(s + 1)
                wd_slabs = []
                for dh in range(2 * ND):
                    wd_s = wd_p.tile([128, NH, 2, 128], bf16, tag="wd")
                    nc.sync.dma_start(wd_s[:], wdd[s, dh])
                    wd_slabs.append(wd_s)
                if s + 1 < NSLOT:
                    slabs_cur = [load_wgu(s + 1, h) for h in range(9)]

                # ---- phase B: stationary = wd d-tiles, moving = hT tokens;
                # 4 consecutive (dh,dt) outputs merge into one store ----
                for qd in range(ND):
                    yst = y_p.tile([128, 4, C], bf16, tag="y")
                    split = (s == NSLOT - 1 and qd == ND - 1)
                    for g4 in range(4):
                        g = qd * 4 + g4
                        dh, dt = g // 2, g % 2
                        pp = psB if g % 2 == 0 else psA
                        py = pp.tile([128, 512], f32,
                                     tag="psB" if pp is psB else "psA")
                        for h in range(NH):
                            nc.tensor.matmul(py[:, :C], wd_slabs[dh][:, h, dt],
                                             hT[:, h, :],
                                             start=(h == 0), stop=(h == NH - 1))
                        nc.vector.tensor_copy(yst[:, g4], py[:, :C])
                        if split:
                            nc.scalar.dma_start(y_out[s][qd, :, g4], yst[:, g4])
                    if not split:
                        nc.scalar.dma_start(y_out[s][qd], yst[:])
                # late slabs for the next slot (gates clear in A(s+1))
                if s + 1 < NSLOT:
                    slabs_cur = slabs_cur + [load_wgu(s + 1, h)
                                             for h in range(9, NH)]
    nc.compile()
    return nc


def _wgu_pack(g, u):
    """[D,Hc] gate/up -> [NH, 128, 2, NKD, 128] slab layout."""
    return (np.stack([g, u]).reshape(2, NKD, 128, NH, 128)
            .transpose(3, 2, 0, 1, 4).astype(BF16))


def _wdd_pack(w):
    """[Hc,D] down -> [2*ND, 128, NH, 2, 128] slab layout."""
    return (w.reshape(NH, 128, 2 * ND, 2, 128)
            .transpose(2, 1, 0, 3, 4).astype(BF16))


def _xgt_pack(xg, C):
    """[D,cnt] gathered tokens -> [128, NKD, C] chunk layout."""
    cnt = xg.shape[1]
    out = np.zeros((128, NKD, C), BF16)
    out[:, :, :cnt] = xg.reshape(NKD, 128, cnt).transpose(1, 0, 2).astype(BF16)
    return np.ascontiguousarray(out)


def kernel(hidden_states, gate_w, wg, wu, wd, swg, swu, swd):
    global LAST_RESULTS
    x = np.ascontiguousarray(np.asarray(hidden_states, np.float32).reshape(T, D))
    gate_w = np.asarray(gate_w, np.float32)
    wg = np.asarray(wg, np.float32)
    wu = np.asarray(wu, np.float32)
    wd = np.asarray(wd, np.float32)
    swg = np.asarray(swg, np.float32)
    swu = np.asarray(swu, np.float32)
    swd = np.asarray(swd, np.float32)

    # ---- host router ----
    topk_i, topk_w = _route(x, gate_w)
    idx = [np.where((topk_i == e).any(-1))[0] for e in range(E)]
    wts = [(topk_w * (topk_i == e))[idx[e]].sum(-1).astype(np.float32) for e in range(E)]
    cnts = np.array([len(i) for i in idx])
    # bucket experts: slot j on every core serves similarly-loaded experts
    ranked = np.argsort(-cnts, kind="stable")            # expert ids, busiest first
    emap = ranked.reshape(E_LOC, N_CORES)                # emap[j, c] -> expert id
    Cr = [max(16, -(-int(cnts[emap[j]].max()) // 4) * 4) for j in range(E_LOC)]
    # cycle order: routed0, shared, routed1..3 (shared slot at SH_POS)
    Cs = Cr[:SH_POS] + [TQ] + Cr[SH_POS:]
    jmap = list(range(SH_POS)) + [None] + list(range(SH_POS, E_LOC))

    nc = _build_bass(Cs)

    # ---- host shard + layout prep (all DMA sources partition-major) ----
    xT = np.ascontiguousarray(x.T)                      # [D, T] fp32

    in_maps = []
    for c in range(N_CORES):
        qc, hc = c >> 1, c & 1                          # shared (quarter, half)
        wgu_np = np.empty((NSLOT, NH, 128, 2, NKD, 128), BF16)
        wdd_np = np.empty((NSLOT, 2 * ND, 128, NH, 2, 128), BF16)
        imap = {"wgu": wgu_np, "wdd": wdd_np}
        hs = slice(hc * HS_LOC, (hc + 1) * HS_LOC)
        for s in range(NSLOT):
            j = jmap[s]
            if j is None:                               # shared-expert slot
                wgu_np[s] = _wgu_pack(swg[:, hs], swu[:, hs])
                wdd_np[s] = _wdd_pack(swd[hs, :])
                imap[f"xgt{s}"] = _xgt_pack(xT[:, qc * TQ:(qc + 1) * TQ], TQ)
            else:
                e = int(emap[j, c])
                wgu_np[s] = _wgu_pack(wg[e], wu[e])
                wdd_np[s] = _wdd_pack(wd[e])
                imap[f"xgt{s}"] = _xgt_pack(xT[:, idx[e]], Cs[s])
        in_maps.append(imap)

    res = run_bass_kernel_spmd(nc, in_maps, core_ids=list(range(N_CORES)))
    LAST_RESULTS = res

    # ---- host unshard: scatter-add routed outputs, add shared partials ----
    out = np.zeros((T, D), np.float32)
    for c in range(N_CORES):
        qc = c >> 1
        for s in range(NSLOT):
            y = (res.results[c][f"y_out{s}"]            # [ND, 128, 4, Cs[s]] bf16
                 .transpose(0, 2, 1, 3).reshape(D, Cs[s]).astype(np.float32))
            j = jmap[s]
            if j is None:
                out[qc * TQ:(qc + 1) * TQ] += y.T
            else:
                e = int(emap[j, c])
                cnt = int(cnts[e])
                out[idx[e]] += (y[:, :cnt] * wts[e][None, :]).T
    return out.reshape(B, S, D)
